# revision 6
# baseline (speedup 1.0000x reference)
"""BQQ linear inference kernel for 8 Trainium2 NeuronCores.

Math: after activation quantization, the whole BQQ op is linear in the
quantized input, so all four correction terms fold into one weight matrix:

    out[b, (j,m)] = X_int[b, (k,n)] @ W'[(k,n), (j,m)] + bias

where X_int = clip(round(x / act_scale), -127, 127) and W' = act_scale * W
is a pure function of the weights (Y_sign/Z_sign/scales/A) and the global
activation scale, all computed on the host (offline weight folding + act
quantization).  The device kernel per core is a pure streaming GEMM:
  1. DMA x^T (int8, upcast to bf16 on DVE) + W' shard (bf16) in, k-ordered
     with escalating chunk sizes so the GEMM starts as soon as k=0 lands.
  2. bias enters PSUM as a contraction-1 matmul (ones outer bias row) that
     opens each accumulation group.
  3. 128-contraction GEMM accumulating over k in PSUM; the last k-steps run
     bank-by-bank so each bank's epilogue overlaps the remaining matmuls.
  4. PSUM -> SBUF bf16 copies (scalar/vector split), DMA out.

Sharding: tensor-parallel over the j (output block) dim, 4 of 32 j-blocks per
core.  Per-core HBM traffic ~6.5 MB (x 2MB int8 + W 4MB bf16 + out 0.5MB).
"""

import numpy as np
import ml_dtypes

import concourse.bass as bass
import concourse.bacc as bacc
import concourse.mybir as mybir
from concourse.tile import TileContext
from concourse.tile_rust import add_dep_helper
from concourse.bass_utils import run_bass_kernel_spmd

F32 = mybir.dt.float32
BF16 = mybir.dt.bfloat16
I8 = mybir.dt.int8

P_, J, K, M, L, N = 2, 32, 32, 128, 16, 128
B = 512                  # tokens
NCORES = 8
JLOC = J // NCORES       # 4 j-blocks per core
CPJ = JLOC * M           # 512 output cols per core
QMAX = 127.0
# k-slices per DMA chunk, escalating so the GEMM k-loop starts early
CHUNKS = [1, 1, 2, 4, 8, 8, 8]
WARMUP = 64
KSPLIT = 24              # k < KSPLIT: banks interleaved; then bank-by-bank

_CACHE = {}


def _build_bass():
    nc = bacc.Bacc()
    xt_d = nc.declare_dram_parameter("xt8", [N, K * B], I8, isOutput=False)
    w_d = nc.declare_dram_parameter("wgt", [N, K * CPJ], BF16, isOutput=False)
    b_d = nc.declare_dram_parameter("bias", [1, CPJ], BF16, isOutput=False)
    out_d = nc.declare_dram_parameter("out", [B, CPJ], BF16, isOutput=True)

    with TileContext(nc) as tc:
        with tc.tile_pool(name="big", bufs=1) as big, \
             tc.tile_pool(name="sm", bufs=1) as sm, \
             tc.tile_pool(name="ot", bufs=4) as ot, \
             tc.tile_pool(name="psum", bufs=1, space="PSUM") as pp:
            xi8 = big.tile([N, K * B], I8)        # x^T int8
            xbt = big.tile([N, K * B], BF16)      # x^T upcast to bf16
            wt = big.tile([N, K * CPJ], BF16)     # folded weights
            wz = sm.tile([128, 192], BF16)        # zeros for PE warmup
            ones_r = sm.tile([1, 128], BF16)
            bias_t = sm.tile([1, CPJ], BF16)
            nc.vector.memset(wz[:], 0.0)
            nc.vector.memset(ones_r[:], 1.0)

            psums = [pp.tile([128, CPJ], F32, name=f"psum{i}", tag=f"psum{i}")
                     for i in range(4)]
            wps = pp.tile([128, 64], F32, name="wps", tag="wps")

            # Phase A: stream x^T (sync HWDGE ring) and weights (scalar HWDGE
            # ring) in parallel, k-ordered; upcast each x chunk on DVE as it
            # lands.  A long run of slim dummy matmuls paced by the first DMA
            # trigger keeps the PE busy through the HAM window so the GEMM
            # starts at full clock.
            bdma = nc.gpsimd.dma_start(out=bias_t[:], in_=b_d[:])
            k0 = 0
            for ci, nk in enumerate(CHUNKS):
                xsl = slice(k0 * B, (k0 + nk) * B)
                wsl = slice(k0 * CPJ, (k0 + nk) * CPJ)
                dma = nc.sync.dma_start(out=xi8[:, xsl], in_=xt_d[:, xsl])
                nc.scalar.dma_start(out=wt[:, wsl], in_=w_d[:, wsl])
                nc.vector.tensor_copy(out=xbt[:, xsl], in_=xi8[:, xsl])
                if ci == 0:
                    for w in range(WARMUP):
                        mm = nc.tensor.matmul(
                            wps[:], lhsT=wz[:, 0:128],
                            rhs=wz[:, 128:192], start=True, stop=True)
                        add_dep_helper(mm.ins, bdma.ins,
                                       reason="pace PE warmup with bias DMA")
                k0 += nk

            # Phase B: bias opens each accumulation group (contraction-1
            # outer product ones x bias_row), then the GEMM k-loop.  The
            # last K - KSPLIT steps run bank-by-bank so bank bb's epilogue
            # can start while bank bb+1 is still accumulating.
            def mm_step(k, bb, stop):
                nc.tensor.matmul(
                    psums[bb][:],
                    lhsT=xbt[:, k * B + bb * 128:k * B + (bb + 1) * 128],
                    rhs=wt[:, k * CPJ:(k + 1) * CPJ],
                    start=False, stop=stop)

            def epilogue(bb):
                o = ot.tile([128, CPJ], BF16)
                if bb % 2 == 0:
                    nc.scalar.copy(o[:], psums[bb][:])
                else:
                    nc.vector.tensor_copy(out=o[:], in_=psums[bb][:])
                eng = nc.sync if bb % 2 == 0 else nc.scalar
                eng.dma_start(out=out_d[bb * 128:(bb + 1) * 128, :], in_=o[:])

            for bb in range(4):
                nc.tensor.matmul(
                    psums[bb][:], lhsT=ones_r[:], rhs=bias_t[:],
                    start=True, stop=False)
            for k in range(KSPLIT):
                for bb in range(4):
                    mm_step(k, bb, stop=False)
            for bb in range(4):
                for k in range(KSPLIT, K):
                    mm_step(k, bb, stop=(k == K - 1))
                epilogue(bb)
    return nc


def _fold_weights(Y_sign, Z_sign, Y_scale, Z_scale, A):
    """W[j,k,n,m]: everything linear in X folded into one matrix (fp32)."""
    ysc = Y_scale[..., 0, 0].astype(np.float32)      # (p,j,k)
    zsc = Z_scale[..., 0, 0].astype(np.float32)
    a0, a1, a2, a3 = (A[..., i].astype(np.float32) for i in range(4))
    Zs = Z_sign.astype(np.float32)
    Ys = Y_sign.astype(np.float32)
    # out1: sum_{p,l} a0*ysc*zsc * Z[l,n] * Y[m,l]  -> (j,k,n,m)
    t1 = np.einsum('pjkln,pjkml->pjknm', Zs, Ys, optimize=True)
    W = np.einsum('pjk,pjknm->jknm', a0 * ysc * zsc, t1, optimize=True)
    # out2: B_coef[j,k,m] broadcast over n
    Ysum = Ys.sum(-1) * ysc[..., None]               # (p,j,k,m)
    W += np.einsum('pjk,pjkm->jkm', a1, Ysum)[:, :, None, :]
    # out3: sum_p a2*zsc*Zsum[n] broadcast over m
    Zsum = Zs.sum(-2) * zsc[..., None]               # (p,j,k,n)
    W += np.einsum('pjk,pjkn->jkn', a2, Zsum)[:, :, :, None]
    # out4: D_coef[j,k] broadcast over n,m
    W += a3.sum(0)[:, :, None, None]
    return W


def _prepare(inputs):
    x = np.asarray(inputs["input"], dtype=np.float32)
    W = _fold_weights(np.asarray(inputs["Y_sign"], np.float32),
                      np.asarray(inputs["Z_sign"], np.float32),
                      np.asarray(inputs["Y_scale"], np.float32),
                      np.asarray(inputs["Z_scale"], np.float32),
                      np.asarray(inputs["A"], np.float32))
    bias = np.asarray(inputs["bias"], np.float32)

    # activation quantization on host (exact global max/min, RNE round)
    act_scale = max((float(x.max()) - float(x.min())) / (2.0 * QMAX), 1e-8)
    xq = np.clip(np.round(x / act_scale), -QMAX, QMAX)
    W = W * act_scale    # fold act_scale into the weights

    # x^T layout [n, (k, b)], int8
    xt8 = np.ascontiguousarray(
        xq.reshape(B, K, N).transpose(2, 1, 0).reshape(N, K * B)).astype(
            np.int8)

    in_maps = []
    for cid in range(NCORES):
        Wc = W[cid * JLOC:(cid + 1) * JLOC]          # [jl,k,n,m]
        wgt = np.ascontiguousarray(
            Wc.transpose(2, 1, 0, 3).reshape(N, K * CPJ)).astype(
                ml_dtypes.bfloat16)                  # [n, (k, jl, m)]
        bc = np.ascontiguousarray(
            bias[cid * CPJ:(cid + 1) * CPJ].reshape(1, CPJ)).astype(
                ml_dtypes.bfloat16)
        in_maps.append({"xt8": xt8, "wgt": wgt, "bias": bc})
    return in_maps


def _run(inputs, trace=False):
    if "nc" not in _CACHE:
        nc = _build_bass()
        nc.finalize()          # run bacc passes (reg alloc, wait splitting)
        _CACHE["nc"] = nc
    nc = _CACHE["nc"]
    in_maps = _prepare(inputs)
    res = run_bass_kernel_spmd(nc, in_maps, list(range(NCORES)), trace=trace)
    out = np.concatenate([res.results[c]["out"].astype(np.float32)
                          for c in range(NCORES)], axis=1)
    out = out.reshape(1, B, J * M)
    return out, res


def kernel(**inputs) -> np.ndarray:
    out, _ = _run(inputs, trace=False)
    return out


# revision 8
# speedup vs baseline: 1.1880x; 1.1880x over previous
"""BQQ linear inference kernel for 8 Trainium2 NeuronCores.

Math: after activation quantization, the whole BQQ op is linear in the
quantized input, so all four correction terms fold into one weight matrix:

    out[b, (j,m)] = X_int[b, (k,n)] @ W'[(k,n), (j,m)] + bias

where X_int = clip(round(x / act_scale), -127, 127) and W' = act_scale * W
is a pure function of the weights (Y_sign/Z_sign/scales/A) and the global
activation scale, all computed on the host (offline weight folding + act
quantization).  The device kernel per core is a pure streaming GEMM:
  1. DMA x^T (int8, upcast to bf16 on DVE) + W' shard (bf16) in, k-ordered
     with escalating chunk sizes so the GEMM starts as soon as k=0 lands.
  2. bias enters PSUM as a contraction-1 matmul (ones outer bias row) that
     opens each accumulation group.
  3. 128-contraction GEMM accumulating over k in PSUM; the last k-steps run
     bank-by-bank so each bank's epilogue overlaps the remaining matmuls.
  4. PSUM -> SBUF bf16 copies (scalar/vector split), DMA out.

Sharding: tensor-parallel over the j (output block) dim, 4 of 32 j-blocks per
core.  Per-core HBM traffic ~6.5 MB (x 2MB int8 + W 4MB bf16 + out 0.5MB).
"""

import numpy as np
import ml_dtypes

import concourse.bass as bass
import concourse.bacc as bacc
import concourse.mybir as mybir
from concourse.tile import TileContext
from concourse.tile_rust import add_dep_helper
from concourse.bass_utils import run_bass_kernel_spmd

F32 = mybir.dt.float32
BF16 = mybir.dt.bfloat16
I8 = mybir.dt.int8

P_, J, K, M, L, N = 2, 32, 32, 128, 16, 128
B = 512                  # tokens
NCORES = 8
JLOC = J // NCORES       # 4 j-blocks per core
CPJ = JLOC * M           # 512 output cols per core
QMAX = 127.0
# k-slices per DMA chunk, escalating so the GEMM k-loop starts early
CHUNKS = [1, 1, 2, 4, 4, 4, 4, 4, 4, 4]
WARMUP = 64
KSPLIT = 24              # k < KSPLIT: banks interleaved; then bank-by-bank

_CACHE = {}


def _build_bass():
    nc = bacc.Bacc()
    xt_d = nc.declare_dram_parameter("xt8", [N, K * B], I8, isOutput=False)
    w_d = nc.declare_dram_parameter("wgt", [N, K * CPJ], BF16, isOutput=False)
    b_d = nc.declare_dram_parameter("bias", [1, CPJ], BF16, isOutput=False)
    out_d = nc.declare_dram_parameter("out", [B, CPJ], BF16, isOutput=True)

    with TileContext(nc) as tc:
        with tc.tile_pool(name="big", bufs=1) as big, \
             tc.tile_pool(name="sm", bufs=1) as sm, \
             tc.tile_pool(name="ot", bufs=4) as ot, \
             tc.tile_pool(name="psum", bufs=1, space="PSUM") as pp:
            xi8 = big.tile([N, K * B], I8)        # x^T int8
            xbt = big.tile([N, K * B], BF16)      # x^T upcast to bf16
            wt = big.tile([N, K * CPJ], BF16)     # folded weights
            wz = sm.tile([128, 192], BF16)        # zeros for PE warmup
            ones_r = sm.tile([1, 128], BF16)
            bias_t = sm.tile([1, CPJ], BF16)
            nc.vector.memset(wz[:], 0.0)
            nc.vector.memset(ones_r[:], 1.0)

            psums = [pp.tile([128, CPJ], F32, name=f"psum{i}", tag=f"psum{i}")
                     for i in range(4)]
            wps = pp.tile([128, 64], F32, name="wps", tag="wps")

            # Phase A: stream x^T (sync HWDGE ring) and weights (scalar HWDGE
            # ring) in parallel, k-ordered; upcast each x chunk on DVE as it
            # lands.  A long run of slim dummy matmuls paced by the first DMA
            # trigger keeps the PE busy through the HAM window so the GEMM
            # starts at full clock.
            bdma = nc.gpsimd.dma_start(out=bias_t[:], in_=b_d[:])
            k0 = 0
            for ci, nk in enumerate(CHUNKS):
                xsl = slice(k0 * B, (k0 + nk) * B)
                wsl = slice(k0 * CPJ, (k0 + nk) * CPJ)
                dma = nc.sync.dma_start(out=xi8[:, xsl], in_=xt_d[:, xsl])
                nc.scalar.dma_start(out=wt[:, wsl], in_=w_d[:, wsl])
                for kk in range(k0, k0 + nk):
                    nc.vector.tensor_copy(out=xbt[:, kk * B:(kk + 1) * B],
                                          in_=xi8[:, kk * B:(kk + 1) * B])
                if ci == 0:
                    for w in range(WARMUP):
                        mm = nc.tensor.matmul(
                            wps[:], lhsT=wz[:, 0:128],
                            rhs=wz[:, 128:192], start=True, stop=True)
                        add_dep_helper(mm.ins, bdma.ins,
                                       reason="pace PE warmup with bias DMA")
                k0 += nk

            # Phase B: bias opens each accumulation group (contraction-1
            # outer product ones x bias_row), then the GEMM k-loop.  The
            # last K - KSPLIT steps run bank-by-bank so bank bb's epilogue
            # can start while bank bb+1 is still accumulating.
            def mm_step(k, bb, stop):
                nc.tensor.matmul(
                    psums[bb][:],
                    lhsT=xbt[:, k * B + bb * 128:k * B + (bb + 1) * 128],
                    rhs=wt[:, k * CPJ:(k + 1) * CPJ],
                    start=False, stop=stop)

            def epilogue(bb):
                o = ot.tile([128, CPJ], BF16)
                if bb % 2 == 0:
                    nc.scalar.copy(o[:], psums[bb][:])
                else:
                    nc.vector.tensor_copy(out=o[:], in_=psums[bb][:])
                eng = nc.sync if bb % 2 == 0 else nc.scalar
                eng.dma_start(out=out_d[bb * 128:(bb + 1) * 128, :], in_=o[:])

            for bb in range(4):
                nc.tensor.matmul(
                    psums[bb][:], lhsT=ones_r[:], rhs=bias_t[:],
                    start=True, stop=False)
            for k in range(KSPLIT):
                for bb in range(4):
                    mm_step(k, bb, stop=False)
            for bb in range(4):
                for k in range(KSPLIT, K):
                    mm_step(k, bb, stop=(k == K - 1))
                epilogue(bb)
    return nc


def _fold_weights(Y_sign, Z_sign, Y_scale, Z_scale, A):
    """W[j,k,n,m]: everything linear in X folded into one matrix (fp32)."""
    ysc = Y_scale[..., 0, 0].astype(np.float32)      # (p,j,k)
    zsc = Z_scale[..., 0, 0].astype(np.float32)
    a0, a1, a2, a3 = (A[..., i].astype(np.float32) for i in range(4))
    Zs = Z_sign.astype(np.float32)
    Ys = Y_sign.astype(np.float32)
    # out1: sum_{p,l} a0*ysc*zsc * Z[l,n] * Y[m,l]  -> (j,k,n,m)
    t1 = np.einsum('pjkln,pjkml->pjknm', Zs, Ys, optimize=True)
    W = np.einsum('pjk,pjknm->jknm', a0 * ysc * zsc, t1, optimize=True)
    # out2: B_coef[j,k,m] broadcast over n
    Ysum = Ys.sum(-1) * ysc[..., None]               # (p,j,k,m)
    W += np.einsum('pjk,pjkm->jkm', a1, Ysum)[:, :, None, :]
    # out3: sum_p a2*zsc*Zsum[n] broadcast over m
    Zsum = Zs.sum(-2) * zsc[..., None]               # (p,j,k,n)
    W += np.einsum('pjk,pjkn->jkn', a2, Zsum)[:, :, :, None]
    # out4: D_coef[j,k] broadcast over n,m
    W += a3.sum(0)[:, :, None, None]
    return W


def _prepare(inputs):
    x = np.asarray(inputs["input"], dtype=np.float32)
    W = _fold_weights(np.asarray(inputs["Y_sign"], np.float32),
                      np.asarray(inputs["Z_sign"], np.float32),
                      np.asarray(inputs["Y_scale"], np.float32),
                      np.asarray(inputs["Z_scale"], np.float32),
                      np.asarray(inputs["A"], np.float32))
    bias = np.asarray(inputs["bias"], np.float32)

    # activation quantization on host (exact global max/min, RNE round)
    act_scale = max((float(x.max()) - float(x.min())) / (2.0 * QMAX), 1e-8)
    xq = np.clip(np.round(x / act_scale), -QMAX, QMAX)
    W = W * act_scale    # fold act_scale into the weights

    # x^T layout [n, (k, b)], int8
    xt8 = np.ascontiguousarray(
        xq.reshape(B, K, N).transpose(2, 1, 0).reshape(N, K * B)).astype(
            np.int8)

    in_maps = []
    for cid in range(NCORES):
        Wc = W[cid * JLOC:(cid + 1) * JLOC]          # [jl,k,n,m]
        wgt = np.ascontiguousarray(
            Wc.transpose(2, 1, 0, 3).reshape(N, K * CPJ)).astype(
                ml_dtypes.bfloat16)                  # [n, (k, jl, m)]
        bc = np.ascontiguousarray(
            bias[cid * CPJ:(cid + 1) * CPJ].reshape(1, CPJ)).astype(
                ml_dtypes.bfloat16)
        in_maps.append({"xt8": xt8, "wgt": wgt, "bias": bc})
    return in_maps


def _run(inputs, trace=False):
    if "nc" not in _CACHE:
        nc = _build_bass()
        nc.finalize()          # run bacc passes (reg alloc, wait splitting)
        _CACHE["nc"] = nc
    nc = _CACHE["nc"]
    in_maps = _prepare(inputs)
    res = run_bass_kernel_spmd(nc, in_maps, list(range(NCORES)), trace=trace)
    out = np.concatenate([res.results[c]["out"].astype(np.float32)
                          for c in range(NCORES)], axis=1)
    out = out.reshape(1, B, J * M)
    return out, res


def kernel(**inputs) -> np.ndarray:
    out, _ = _run(inputs, trace=False)
    return out
